# revision 24
# baseline (speedup 1.0000x reference)
import sys, os
for _p in ("/opt/trn_rl_repo",):
    if _p not in sys.path:
        sys.path.append(_p)

import numpy as np
import ml_dtypes
from contextlib import ExitStack

import concourse.bass as bass
import concourse.bacc as bacc
import concourse.tile as tile
from concourse import mybir
from concourse.bass_utils import run_bass_kernel_spmd

F32 = mybir.dt.float32
F32R = mybir.dt.float32r
BF16 = mybir.dt.bfloat16
BF_NP = ml_dtypes.bfloat16

DIM = 256
HEADS = 8
DIM_HEAD = 64
SLICE_NUM = 64
INNER = HEADS * DIM_HEAD  # 512
B, N = 4, 32768
NCORES = 8
NSHARD = N // 2  # 16384 tokens per core
P = 128
EXPF = mybir.ActivationFunctionType.Exp
FP8 = mybir.dt.float8e4
FP8_NP = ml_dtypes.float8_e4m3
FP16 = mybir.dt.float16
DR = mybir.MatmulPerfMode.DoubleRow
FX_SCALE = 16.0  # fp8 range scaling for Wfx; compensated in wq/wo/bfx
LG_SHIFT = 5.0   # logit shift (softmax-invariant) so exp sums fit fp16


def build_program(nshard, dbg=False):
    NT = nshard // P
    assert NT % 8 == 0
    NQ = NT // 4  # quad-tiles for pass 2
    nc = bacc.Bacc("TRN2", target_bir_lowering=False, debug=False,
                   num_devices=NCORES)
    if dbg:
        dbg_pooled = nc.dram_tensor("dbg_pooled", [P, 4, 130], F32,
                                    kind="ExternalOutput").ap()
        dbg_m2 = nc.dram_tensor("dbg_m2", [P, 4, DIM], BF16,
                                kind="ExternalOutput").ap()
        dbg_wT = nc.dram_tensor("dbg_wT", [P, 4, nshard], BF16,
                                kind="ExternalOutput").ap()
    xT_h = nc.dram_tensor("xT", [DIM, nshard], F32R, kind="ExternalInput")
    x8T_h = nc.dram_tensor("x8T", [DIM, nshard], FP8, kind="ExternalInput")
    wfx8T = nc.dram_tensor("wfx8T", [DIM, INNER], FP8, kind="ExternalInput")
    wlgT = nc.dram_tensor("wlgT", [DIM, INNER], F32R, kind="ExternalInput").ap()
    blgb = nc.dram_tensor("blgb", [P, INNER], F32R, kind="ExternalInput").ap()
    idf32r = nc.dram_tensor("idf32r", [P, P], F32R, kind="ExternalInput").ap()
    bfxb = nc.dram_tensor("bfxb", [P, 4, 64], F32, kind="ExternalInput").ap()
    wqT = nc.dram_tensor("wqT", [64, 64], F32, kind="ExternalInput").ap()
    wkT = nc.dram_tensor("wkT", [64, 64], F32, kind="ExternalInput").ap()
    wvT = nc.dram_tensor("wvT", [64, 64], F32, kind="ExternalInput").ap()
    woT = nc.dram_tensor("woT", [64, HEADS, DIM], F32, kind="ExternalInput").ap()
    boutT = nc.dram_tensor("boutT", [P, 2], F32, kind="ExternalInput").ap()
    idbf = nc.dram_tensor("idbf", [P, P], BF16, kind="ExternalInput").ap()
    idf32 = nc.dram_tensor("idf32", [P, P], F32, kind="ExternalInput").ap()
    out_ap = nc.dram_tensor("outT", [DIM, nshard], F32, kind="ExternalOutput").ap()
    xT = xT_h.ap()

    with tile.TileContext(nc) as tc, ExitStack() as ctx:
        cpool = ctx.enter_context(tc.tile_pool(name="consts", bufs=1))
        big = ctx.enter_context(tc.tile_pool(name="big", bufs=1))
        xpool = ctx.enter_context(tc.tile_pool(name="xp", bufs=3))
        x8pool = ctx.enter_context(tc.tile_pool(name="x8p", bufs=3))

        # first x tile group goes out before any weight DMA (longest pole).
        GW = 2 * P  # tokens per x group
        def x_group_load(g):
            xt = xpool.tile([P, 2, GW], F32R)
            src = bass.AP(xT_h, g * GW,
                          [[nshard, P], [P * nshard, 2], [1, GW]])
            nc.sync.dma_start(xt[:], src)
            x8 = x8pool.tile([P, 2, GW], FP8)
            src8 = bass.AP(x8T_h, g * GW,
                           [[nshard, P], [P * nshard, 2], [1, GW]])
            nc.sync.dma_start(x8[:], src8)
            return xt, x8

        xt0, x8t0 = x_group_load(0)

        # weights needed by pass-1 matmuls next on the sync queue
        wfx8_sb = cpool.tile([P, 2, INNER], FP8)
        nc.sync.dma_start(wfx8_sb[:],
                          bass.AP(wfx8T, 0, [[INNER, P], [P * INNER, 2],
                                             [1, INNER]]))
        wlg_sb = cpool.tile([P, 2, INNER], F32R)
        for c in range(2):
            nc.sync.dma_start(wlg_sb[:, c, :], wlgT[c * P:(c + 1) * P, :])
        idb_sb = cpool.tile([P, P], BF16)
        nc.sync.dma_start(idb_sb[:], idbf[:])
        blg_sb = cpool.tile([P, INNER], F32R)
        nc.sync.dma_start(blg_sb[:], blgb[:])
        idr_sb = cpool.tile([P, P], F32R)
        nc.sync.dma_start(idr_sb[:], idf32r[:])
        # everything else (attention/pass-2 consts) via the idle gpsimd queue
        bfx_sb = cpool.tile([P, 4, 64], F32)
        nc.gpsimd.dma_start(bfx_sb[:], bfxb[:])
        wq_sb = cpool.tile([64, 64], F32)
        wk_sb = cpool.tile([64, 64], F32)
        wv_sb = cpool.tile([64, 64], F32)
        nc.gpsimd.dma_start(wq_sb[:], wqT[:])
        nc.gpsimd.dma_start(wk_sb[:], wkT[:])
        nc.gpsimd.dma_start(wv_sb[:], wvT[:])
        wo_sb = cpool.tile([64, HEADS, DIM], F32)
        nc.gpsimd.dma_start(wo_sb[:], woT[:])
        boutT_sb = cpool.tile([P, 2], F32)
        nc.gpsimd.dma_start(boutT_sb[:], boutT[:])
        idf_sb = cpool.tile([P, P], F32)
        nc.gpsimd.dma_start(idf_sb[:], idf32[:])

        # persistent across phases
        wT_sb = big.tile([P, 4, nshard], BF16)   # transposed slice weights
        pooled_sb = big.tile([P, 4, 130], F32)   # after allreduce
        m2_sb = big.tile([P, 4, DIM], BF16)      # out_slice @ WoutT per hg

        # persistent fx staging tiles (cols 128:130 stay 1.0 after one memset)
        fxe = [big.tile([P, 4, 130], BF16, name=f"fxe{i}") for i in range(3)]
        for i in range(3):
            nc.vector.memset(fxe[i][:, :, 128:130], 1.0)

        # dummy warm-up collective: pays the CC mesh setup + pair sync at
        # kernel start (overlapping pass-1) instead of on the critical path
        wdram = ctx.enter_context(tc.tile_pool(name="wdram", bufs=1,
                                               space="DRAM"))
        wu_in = wdram.tile([1, 2], F32)
        wu_out = wdram.tile([1, 2], F32)
        wu_sb = cpool.tile([1, 2], F32)
        nc.gpsimd.memset(wu_sb[:], 0.0)
        nc.gpsimd.dma_start(wu_in[:], wu_sb[:])
        nc.gpsimd.collective_compute(
            "AllReduce", mybir.AluOpType.add,
            replica_groups=[[0, 1], [2, 3], [4, 5], [6, 7]],
            ins=[wu_in.opt()], outs=[wu_out.opt()])

        # ---------------- pass 1 ----------------
        # Software-pipelined by one tile: the pool/transpose matmuls for
        # tile t-1 are emitted after tile t's main matmuls, so the PE never
        # blocks on the ~2us ACT->DVE softmax chain that produces w_t.
        with tc.tile_pool(name="sp", bufs=4) as spool, \
             tc.tile_pool(name="fxps", bufs=2, space="PSUM") as fxps, \
             tc.tile_pool(name="lgps", bufs=1, space="PSUM") as lgps, \
             tc.tile_pool(name="tps", bufs=1, space="PSUM") as tps, \
             tc.tile_pool(name="poolps", bufs=1, space="PSUM") as poolps:
            # one PSUM bank per accumulator: accumulation groups must not
            # share a bank (start=True resets the bank's accum state)
            pool_ps = [poolps.tile([P, 130], F32, name=f"pool_ps{i}")
                       for i in range(4)]

            def pool_and_transpose(t, w_t, fx_sb):
                for q in range(4):
                    nc.tensor.matmul(pool_ps[q][:],
                                     w_t[:, 2 * q:2 * q + 2, :],
                                     fx_sb[:, q, :],
                                     start=(t == 0), stop=(t == NT - 1))
                wtp = tps.tile([P, 4, P], BF16)
                for c in range(4):
                    nc.tensor.transpose(wtp[:, c, :],
                                        w_t[:, 2 * c:2 * c + 2, :],
                                        idb_sb[:])
                nc.vector.tensor_copy(wT_sb[:, 0:4, t * P:(t + 1) * P],
                                      wtp[:])

            prev = None
            for g in range(NT // 2):
                xt, x8t = (xt0, x8t0) if g == 0 else x_group_load(g)
                for s in range(2):
                    t = 2 * g + s
                    xa = xt[:, 0, s * P:(s + 1) * P]
                    xb = xt[:, 1, s * P:(s + 1) * P]
                    fxp = fxps.tile([P, 4, P], F32)
                    nc.tensor.matmul(fxp[:], x8t[:, :, s * P:(s + 1) * P],
                                     wfx8_sb[:], start=True, stop=True,
                                     perf_mode=DR)
                    lgp = lgps.tile([P, HEADS, SLICE_NUM], F32)
                    nc.tensor.matmul(lgp[:], idr_sb[:], blg_sb[:],
                                     start=True, stop=False)
                    nc.tensor.matmul(lgp[:], xa, wlg_sb[:, 0, :],
                                     start=False, stop=False)
                    nc.tensor.matmul(lgp[:], xb, wlg_sb[:, 1, :],
                                     start=False, stop=True)
                    if prev is not None:
                        pool_and_transpose(*prev)
                    # softmax over slices; logits pre-shifted by -LG_SHIFT
                    # (softmax-invariant) so per-head exp sums fit fp16
                    e_t = spool.tile([P, HEADS, SLICE_NUM], BF16)
                    nc.scalar.activation(e_t[:], lgp[:], EXPF)
                    s_t = spool.tile([P, HEADS], FP16)
                    with nc.allow_low_precision(
                            reason="exp sums bounded by logit shift"):
                        nc.vector.tensor_reduce(s_t[:], e_t[:],
                                                axis=mybir.AxisListType.X,
                                                op=mybir.AluOpType.add)
                    r_t = spool.tile([P, HEADS], F32)
                    nc.vector.reciprocal(r_t[:], s_t[:])
                    w_t = spool.tile([P, HEADS, SLICE_NUM], BF16)
                    weng = nc.vector if t % 2 == 0 else nc.gpsimd
                    weng.tensor_mul(
                        w_t[:], e_t[:],
                        r_t[:, :, None].to_broadcast([P, HEADS, SLICE_NUM]))
                    fx_sb = fxe[t % 3]
                    nc.scalar.copy(fx_sb[:, :, 0:128], fxp[:])
                    prev = (t, w_t, fx_sb)
            pool_and_transpose(*prev)

            # -------- allreduce pooled sums over the token-half pair --------
            with tc.tile_pool(name="ccdram", bufs=1, space="DRAM") as dpool:
                b_in = dpool.tile([P, 4, 130], F32)
                b_out = dpool.tile([P, 4, 130], F32)
                pre_sb = big.tile([P, 4, 130], F32)
                for q in range(4):
                    nc.vector.tensor_copy(pre_sb[:, q, :], pool_ps[q][:])
                nc.sync.dma_start(b_in[:], pre_sb[:])
                nc.gpsimd.collective_compute(
                    "AllReduce", mybir.AluOpType.add,
                    replica_groups=[[0, 1], [2, 3], [4, 5], [6, 7]],
                    ins=[b_in.opt()], outs=[b_out.opt()])
                nc.sync.dma_start(pooled_sb[:], b_out[:])

        # ---------------- tiny slice attention (head pairs, 128-wide) ----
        with tc.tile_pool(name="mps", bufs=1, space="PSUM") as mps, \
             tc.tile_pool(name="msb", bufs=2) as msb:
            for q4 in range(4):
                norm = pooled_sb[:, q4, 128:129]
                nrm = msb.tile([P, 1], F32)
                nc.vector.tensor_scalar_add(nrm[:], norm, 1e-5)
                rho = msb.tile([P, 1], F32)
                nc.vector.reciprocal(rho[:], nrm[:])
                tmp = msb.tile([P, 64], F32)
                nc.vector.tensor_scalar_mul(tmp[:], bfx_sb[:, q4, :], norm)
                # diagonal S blocks read in place: row j*64+g <- own head
                stp = msb.tile([P, 64], F32)
                for j in range(2):
                    nc.vector.tensor_add(
                        stp[j * 64:(j + 1) * 64, :],
                        pooled_sb[j * 64:(j + 1) * 64, q4,
                                  j * 64:j * 64 + 64],
                        tmp[j * 64:(j + 1) * 64, :])
                st = msb.tile([P, 64], F32)
                nc.vector.tensor_scalar_mul(st[:], stp[:], rho[:])
                # stT [c, j*64+g] = slice_token[head j, g, c]
                stT_p = mps.tile([64, P], F32)
                nc.tensor.transpose(stT_p[:], st[:], idf_sb[:])
                stT = msb.tile([64, P], F32)
                nc.scalar.copy(stT[:], stT_p[:])
                # q^T, k^T for both heads at once: [o, j*64+g]
                qk_p = mps.tile([64, 2, P], F32)
                nc.tensor.matmul(qk_p[:, 0, :], wq_sb[:], stT[:],
                                 start=True, stop=True)
                nc.tensor.matmul(qk_p[:, 1, :], wk_sb[:], stT[:],
                                 start=True, stop=True)
                qk = msb.tile([64, 2, P], F32)
                nc.scalar.copy(qk[:], qk_p[:])
                # logits per head -> stacked [j*64+g, g']
                L_p = mps.tile([P, 64], F32)
                for j in range(2):
                    nc.tensor.matmul(L_p[j * 64:(j + 1) * 64, :],
                                     qk[:, 0, j * 64:(j + 1) * 64],
                                     qk[:, 1, j * 64:(j + 1) * 64],
                                     start=True, stop=True)
                ea = msb.tile([P, 64], F32)
                srow = msb.tile([P, 1], F32)
                nc.scalar.activation(ea[:], L_p[:], EXPF, accum_out=srow[:])
                rha = msb.tile([P, 1], F32)
                nc.vector.reciprocal(rha[:], srow[:])
                attn = msb.tile([P, 64], F32)
                nc.vector.tensor_scalar_mul(attn[:], ea[:], rha[:])
                # aT [g, j*64+g'] = attn[head j, g', g]
                aT_p = mps.tile([64, P], F32)
                nc.tensor.transpose(aT_p[:], attn[:], idf_sb[:])
                aT = msb.tile([64, P], F32)
                nc.scalar.copy(aT[:], aT_p[:])
                # v per head [g, o] (base partition 0), then os = attn @ v
                os_p = mps.tile([P, 64], F32)
                for j in range(2):
                    v_p = mps.tile([64, 64], F32)
                    nc.tensor.matmul(v_p[:], stT[:, j * 64:(j + 1) * 64],
                                     wv_sb[:], start=True, stop=True)
                    v_sb = msb.tile([64, 64], F32)
                    nc.scalar.copy(v_sb[:], v_p[:])
                    nc.tensor.matmul(os_p[j * 64:(j + 1) * 64, :],
                                     aT[:, j * 64:(j + 1) * 64], v_sb[:],
                                     start=True, stop=True)
                os_sb = msb.tile([P, 64], F32)
                nc.scalar.copy(os_sb[:], os_p[:])
                osT_p = mps.tile([64, P], F32)
                nc.tensor.transpose(osT_p[:], os_sb[:], idf_sb[:])
                osT = msb.tile([64, P], F32)
                nc.scalar.copy(osT[:], osT_p[:])
                m2_p = mps.tile([P, DIM], F32)
                for j in range(2):
                    nc.tensor.matmul(m2_p[j * 64:(j + 1) * 64, :],
                                     osT[:, j * 64:(j + 1) * 64],
                                     wo_sb[:, 2 * q4 + j, :],
                                     start=True, stop=True)
                nc.scalar.copy(m2_sb[:, q4, :], m2_p[:])

        # -------- pass 2: unpool + output proj, transposed output --------
        # outT[d, tok] = sum_c m2[:, c, d].T @ wT[:, c, tok]; m2 stationary.
        P2W = 512  # tokens per psum group (one bank per tile)
        NP2 = nshard // P2W
        with tc.tile_pool(name="p2ps", bufs=4, space="PSUM") as p2ps, \
             tc.tile_pool(name="p2sb", bufs=4) as p2sb:
            for qt in range(NP2):
                for half in range(2):
                    op = p2ps.tile([P, P2W], F32)
                    for c in range(4):
                        nc.tensor.matmul(
                            op[:],
                            m2_sb[:, c, half * P:(half + 1) * P],
                            wT_sb[:, c, qt * P2W:(qt + 1) * P2W],
                            start=(c == 0), stop=(c == 3))
                    ob = p2sb.tile([P, P2W], F32)
                    nc.vector.tensor_scalar_add(ob[:], op[:],
                                                boutT_sb[:, half:half + 1])
                    nc.sync.dma_start(
                        out_ap[half * P:(half + 1) * P,
                               qt * P2W:(qt + 1) * P2W], ob[:])
        if dbg:
            nc.sync.dma_start(dbg_pooled[:], pooled_sb[:])
            nc.sync.dma_start(dbg_m2[:], m2_sb[:])
            nc.sync.dma_start(dbg_wT[:], wT_sb[:])
    nc.compile()
    return nc


def _bfx_pair(bfx):
    bfx2 = bfx.reshape(HEADS, DIM_HEAD)
    out = np.empty((P, 4, 64), np.float32)
    for q4 in range(4):
        for j in range(2):
            out[j * 64:(j + 1) * 64, q4, :] = bfx2[2 * q4 + j]
    return out


def prep_weights(inputs):
    f32 = np.float32
    Wfx = np.asarray(inputs["Wfx"], f32)
    bfx = np.asarray(inputs["bfx"], f32)
    Wx = np.asarray(inputs["Wx"], f32)
    bx = np.asarray(inputs["bx"], f32)
    Wslice = np.asarray(inputs["Wslice"], f32)
    bslice = np.asarray(inputs["bslice"], f32)
    tau = np.asarray(inputs["temperature"], f32).reshape(HEADS)
    Wq = np.asarray(inputs["Wq"], f32)
    Wk = np.asarray(inputs["Wk"], f32)
    Wv = np.asarray(inputs["Wv"], f32)
    Wout = np.asarray(inputs["Wout"], f32)
    bout = np.asarray(inputs["bout"], f32)

    wlg_blocks = []
    blg_blocks = []
    for h in range(HEADS):
        Wx_h = Wx[h * DIM_HEAD:(h + 1) * DIM_HEAD, :]
        bx_h = bx[h * DIM_HEAD:(h + 1) * DIM_HEAD]
        wlg_blocks.append((Wslice @ Wx_h) / tau[h])
        blg_blocks.append((Wslice @ bx_h + bslice) / tau[h])
    wlgT = np.ascontiguousarray(np.concatenate(wlg_blocks, 0).T, f32)
    blg = np.concatenate(blg_blocks, 0).reshape(1, INNER).astype(f32)
    scale = DIM_HEAD ** -0.5
    # fx path carries FX_SCALE from the fp8 weights; undo it in the
    # slice-attention weights: q@k picks up FX_SCALE^2, v@wo picks up FX_SCALE
    return {
        "wfx8T": np.ascontiguousarray(Wfx.T * FX_SCALE).astype(FP8_NP),
        "wlgT": wlgT,
        "blgb": np.ascontiguousarray(np.tile(blg - LG_SHIFT, (P, 1)), f32),
        "idf32r": np.eye(P, dtype=f32),
        "bfxb": _bfx_pair(bfx) * FX_SCALE,
        "wqT": np.ascontiguousarray((Wq * (scale / FX_SCALE ** 2)).T, f32),
        "wkT": np.ascontiguousarray(Wk.T, f32),
        "wvT": np.ascontiguousarray(Wv.T, f32),
        "woT": np.ascontiguousarray(
            Wout.T.reshape(HEADS, DIM_HEAD, DIM).transpose(1, 0, 2),
            f32) / FX_SCALE,
        "boutT": np.ascontiguousarray(bout.reshape(2, P).T, f32),
        "idbf": np.eye(P, dtype=BF_NP),
        "idf32": np.eye(P, dtype=np.float32),
    }


_PROG = {}


def _get_prog(nshard, dbg=False):
    if (nshard, dbg) not in _PROG:
        _PROG[(nshard, dbg)] = build_program(nshard, dbg)
    return _PROG[(nshard, dbg)]


def run(inputs, nshard=NSHARD, trace=False, trace_cores=None, dbg=False):
    x = np.asarray(inputs["x"], np.float32)
    b_, n_, d_ = x.shape
    assert d_ == DIM and n_ == 2 * nshard and b_ == B
    nc = _get_prog(nshard, dbg)
    common = prep_weights(inputs)
    in_maps = []
    for core in range(NCORES):
        bb, half = core // 2, core % 2
        xs = x[bb, half * nshard:(half + 1) * nshard, :]
        m = dict(common)
        xT = np.ascontiguousarray(xs.T)
        m["xT"] = xT
        m["x8T"] = xT.astype(FP8_NP)
        in_maps.append(m)
    res = run_bass_kernel_spmd(nc, in_maps, list(range(NCORES)),
                               trace=trace, trace_cores=trace_cores)
    full = np.empty((B, n_, DIM), np.float32)
    for core in range(NCORES):
        bb, half = core // 2, core % 2
        full[bb, half * nshard:(half + 1) * nshard, :] = \
            res.results[core]["outT"].T
    return full, res


def kernel(**inputs):
    out, _ = run(inputs)
    return out
